# revision 14
# baseline (speedup 1.0000x reference)
"""Trainium2 Bass kernel for nn_DPR (dense_transformer).

Distribution: batch-parallel over 8 NeuronCores. The reference's
`.view`-emulating reshapes scramble the batch axis into the attention
contraction, so after the per-core q projection each core needs a specific
p-slice of every other core's q. That exchange is one AllToAll of
[8, 8*64*192] bf16 per core (~1.5 MB); attention, softmax, and the output
head are then batch-local.

Math (validated against the jax reference in numpy):
  q[b,p,d]   = sum_n (Wq[p,n]/TEMP) query[b,d,n]          (1/TEMP folded in)
  kf         = concat([mean(k5), k5]).reshape(512, 1152)   (k mean via linearity)
  attn       = qf.T @ kf ; qf[r, m] = q[r//8, (r%8)*64 + m//192, m%192]
  softmax over each 192-column segment of each attn row
  out_pre[m,s] = sum_j E[m,s*192+j] * v[m//32, j] / sumexp[m,s]
  out2       = out_pre.reshape(48, 192) @ Wf.T  (per-core g block)
  out        = layernorm(out2 + proto) * ln_g + ln_b

Precision: all projection/attention matmuls run in bf16 (fp32 PSUM accum),
validated to ~3.5e-4 end-to-end rel err in numpy vs the 2e-2 gate. The
weight operands (Wq, Wk, support) are cast to bf16 once and transposed via
the DMA XBAR (InstDmaTransposeAnt, 2-byte dtype, zero PE cost). query is
transposed on the PE from fp32 (2 cyc/row) with the bf16 cast folded into
the PSUM evacuation. Softmax/PV/output head stay fp32/f32r.
"""
import os
import numpy as np
from contextlib import ExitStack

import concourse.bass as bass
import concourse.tile as tile
from concourse import mybir, bacc
from concourse.bass_utils import run_bass_kernel_spmd
from concourse.masks import make_identity

F32 = mybir.dt.float32
F32R = mybir.dt.float32r
BF16 = mybir.dt.bfloat16
ExpF = mybir.ActivationFunctionType.Exp
SqrtF = mybir.ActivationFunctionType.Sqrt
AX = mybir.AxisListType.X
MULT = mybir.AluOpType.mult
ADD = mybir.AluOpType.add

NCORES = 8
FEAT = 192          # d
PTS = 2048          # n
PROJ = 512          # p
BATCH = 64
NSUP = 5
WAY = 6
BL = BATCH // NCORES            # 8 local batches
GL = BL * WAY                   # 48 local (b, w) rows
TEMP = float(FEAT) ** 0.5
LN_EPS = 1e-5
NKT = PTS // 128                # 16 contraction tiles for projections
MT = BL * FEAT // 128           # 12 attention row tiles (1536 rows)
NCOL = WAY * FEAT               # 1152 attention cols
QCH = BL * 64 * FEAT            # 98304 q elems per A2A chunk
KCH = 128 * 576                 # 73728 k_part elems appended to each chunk
QCH2 = QCH + KCH                # 172032 total chunk elems
NS = NSUP * FEAT                # 960


def _ap(t, offset, dims):
    return bass.AP(tensor=t.tensor, offset=t.offset + offset, ap=list(dims))


def build(repeat=None):
    if repeat is None:
        repeat = int(os.environ.get("KERNEL_REPEAT", "1"))
    skip_cc = bool(os.environ.get("KERNEL_SKIP_CC"))
    nc = bacc.Bacc("TRN2", target_bir_lowering=False, debug=False,
                   num_devices=NCORES)

    query = nc.dram_tensor("query", [BL, FEAT, PTS], F32, kind="ExternalInput").ap()
    sup_sh = nc.dram_tensor("sup_sh", [3, FEAT, PTS], F32, kind="ExternalInput").ap()
    proto = nc.dram_tensor("proto", [GL, FEAT], F32, kind="ExternalInput").ap()
    Wq = nc.dram_tensor("Wq", [PROJ, PTS], F32, kind="ExternalInput").ap()
    Wk_sh = nc.dram_tensor("Wk_sh", [128, PTS], F32, kind="ExternalInput").ap()
    Wv = nc.dram_tensor("Wv", [FEAT, FEAT], F32, kind="ExternalInput").ap()
    bv = nc.dram_tensor("bv", [FEAT], F32, kind="ExternalInput").ap()
    Wf = nc.dram_tensor("Wf", [FEAT, FEAT], F32, kind="ExternalInput").ap()
    ln_g = nc.dram_tensor("ln_g", [FEAT], F32, kind="ExternalInput").ap()
    ln_b = nc.dram_tensor("ln_b", [FEAT], F32, kind="ExternalInput").ap()
    out_l = nc.dram_tensor("out_l", [GL, FEAT], F32, kind="ExternalOutput").ap()

    a2a_in = nc.dram_tensor("a2a_in", [NCORES, QCH], BF16)
    a2a_out = nc.dram_tensor("a2a_out", [NCORES, QCH], BF16)
    kflat = nc.dram_tensor("kflat", [WAY * PROJ, FEAT], BF16)
    kag_in = nc.dram_tensor("kag_in", [128 * 576], BF16)
    kag_out = nc.dram_tensor("kag_out", [NCORES, 128 * 576], BF16, addr_space="Shared")
    vflat = nc.dram_tensor("vflat", [GL, FEAT], F32)
    opre_d = nc.dram_tensor("opre_d", [MT * 128, WAY], F32)
    # guard sink: tiny DMAs through here gate the XBAR-transpose triggers on
    # their queue, because the XBAR instruction's own wait field is not
    # honored by the ucode DMA path (races with the producing cast otherwise)
    sink = nc.dram_tensor("sink", [4096], BF16)

    with tile.TileContext(nc) as tc:
        for rep in range(repeat):
            _emit(nc, tc, rep, skip_cc,
                  query, sup_sh, proto, Wq, Wk_sh, Wv, bv, Wf, ln_g, ln_b,
                  out_l, a2a_in, a2a_out, kflat, vflat, opre_d,
                  kag_in, kag_out, sink)

    nc.compile()
    return nc


def _emit(nc, tc, rep, skip_cc,
          query, sup_sh, proto, Wq, Wk_sh, Wv, bv, Wf, ln_g, ln_b,
          out_l, a2a_in, a2a_out, kflat, vflat, opre_d,
          kag_in, kag_out, sink):
    guard_off = [0]

    def _guard(eng, src_slice):
        eng.dma_start(_ap(sink.ap(), guard_off[0], [[1, 4]]), src_slice)
        guard_off[0] = (guard_off[0] + 4) % 4096
    R = f"r{rep}"
    with ExitStack() as top:
        const = top.enter_context(tc.tile_pool(name=f"const{R}", bufs=1))
        ident = const.tile([128, 128], F32, name=f"ident{R}")
        make_identity(nc, ident)
        proto_sb = const.tile([GL, FEAT], F32, name=f"proto_sb{R}")
        nc.scalar.dma_start(proto_sb, proto)

        # ============ v = proto @ Wv.T + bv (tiny, local) ============
        with ExitStack() as ctx:
            vp = ctx.enter_context(tc.tile_pool(name=f"vp{R}", bufs=1))
            vps = ctx.enter_context(tc.tile_pool(name=f"vps{R}", bufs=2, space="PSUM"))

            wv_hi = vp.tile([128, FEAT], F32, tag="wn")
            wv_lo = vp.tile([64, FEAT], F32, tag="wn2")
            nc.scalar.dma_start(wv_hi, Wv[0:128, :])
            nc.scalar.dma_start(wv_lo, Wv[128:192, :])

            wvT_hi = vp.tile([128, FEAT], F32R, tag="wt")
            wvT_lo = vp.tile([64, FEAT], F32R, tag="wt2")
            ptT_hi = vp.tile([128, GL], F32R, tag="pt")
            ptT_lo = vp.tile([64, GL], F32R, tag="pt2")
            for (dst, dcol, src) in (
                (wvT_hi, slice(0, 128), wv_hi[:, 0:128]),
                (wvT_lo, slice(0, 128), wv_hi[:, 128:192]),
                (wvT_hi, slice(128, 192), wv_lo[:, 0:128]),
                (wvT_lo, slice(128, 192), wv_lo[:, 128:192]),
                (ptT_hi, slice(0, GL), proto_sb[:, 0:128]),
                (ptT_lo, slice(0, GL), proto_sb[:, 128:192]),
            ):
                p_in, f_in = src.shape
                ps_t = vps.tile([128, 128], F32, tag="tp")
                nc.tensor.transpose(ps_t[:f_in, :p_in], src, ident[:p_in, :p_in])
                nc.vector.tensor_copy(dst[:f_in, dcol], ps_t[:f_in, :p_in])

            ps_v = vps.tile([GL, FEAT], F32, tag="v")
            nc.tensor.matmul(ps_v, ptT_hi, wvT_hi, start=True, stop=False)
            nc.tensor.matmul(ps_v, ptT_lo, wvT_lo, start=False, stop=True)
            bv_bc = vp.tile([GL, FEAT], F32, tag="bv")
            nc.scalar.dma_start(bv_bc, _ap(bv, 0, [[0, GL], [1, FEAT]]))
            v_sb = vp.tile([GL, FEAT], F32, tag="vs")
            nc.vector.tensor_add(v_sb, ps_v, bv_bc)
            nc.scalar.dma_start(vflat.ap(), v_sb)

        # ====== k projection: 8-way sharded (pt = core%4, ways = core//4) ======
        # Each core projects its [128] p-block against its 3 support ways
        # (half1 cores carry a zero third way), AllGathers the pieces, and
        # reassembles full kflat locally. The AG overlaps the q phase.
        # Wk/support are cast to bf16 and transposed on the DMA XBAR.
        with ExitStack() as ctxk:
            wkp = ctxk.enter_context(tc.tile_pool(name=f"wkp{R}", bufs=1))
            wkb = ctxk.enter_context(tc.tile_pool(name=f"wkb{R}", bufs=1))
            sn = ctxk.enter_context(tc.tile_pool(name=f"sn{R}", bufs=2))
            sb6 = ctxk.enter_context(tc.tile_pool(name=f"sb6{R}", bufs=1))
            kps = ctxk.enter_context(tc.tile_pool(name=f"kps{R}", bufs=1, space="PSUM"))
            kevac = ctxk.enter_context(tc.tile_pool(name=f"kevac{R}", bufs=1))

            wk_nat = wkp.tile([128, PTS], F32, tag="wkn")
            nc.sync.dma_start(wk_nat, Wk_sh)
            wk_b = wkb.tile([128, PTS], BF16, tag="wkb")
            nc.vector.tensor_copy(wk_b, wk_nat)
            wkT = wkb.tile([128, NKT, 128], BF16, tag="wkT")
            _guard(nc.sync, wk_b[0:1, 0:4])
            nc.sync.dma_start_transpose(wkT[:, :, :], wk_b)

            # XBAR transposes need contiguous destinations (strided slices
            # produce wrong output on hardware) -> way-major tiles.
            supT_hi = sb6.tile([128, 3, NKT, 128], BF16, tag="supTh")
            supT_lo = sb6.tile([128, 3, NKT, 64], BF16, tag="supTl")
            for wi in range(3):
                hi = sn.tile([128, PTS], F32, tag="shi", name=f"shi{R}{wi}")
                lo = sn.tile([64, PTS], F32, tag="slo", name=f"slo{R}{wi}")
                nc.sync.dma_start(hi, sup_sh[wi, 0:128, :])
                nc.sync.dma_start(lo, sup_sh[wi, 128:192, :])
                hi_b = sn.tile([128, PTS], BF16, tag="shib", name=f"shib{R}{wi}")
                lo_b = sn.tile([64, PTS], BF16, tag="slob", name=f"slob{R}{wi}")
                nc.scalar.copy(hi_b, hi)
                nc.vector.tensor_copy(lo_b, lo)
                _guard(nc.sync, hi_b[0:1, 0:4])
                nc.sync.dma_start_transpose(supT_hi[:, wi, :, :], hi_b)
                _guard(nc.scalar, lo_b[0:1, 0:4])
                nc.scalar.dma_start_transpose(supT_lo[:, wi, :, :], lo_b)

            # one accumulation group per PSUM bank: interleaved groups
            # sharing a bank corrupt each other's accumulation
            ps_hi = [kps.tile([128, 128], F32, tag=f"kh{wi}",
                              name=f"kh{R}{wi}") for wi in range(3)]
            ps_lo = [kps.tile([128, 64], F32, tag=f"kl{wi}",
                              name=f"kl{R}{wi}") for wi in range(3)]
            for kt in range(NKT):
                st, sp = (kt == 0), (kt == NKT - 1)
                for wi in range(3):
                    nc.tensor.matmul(ps_hi[wi], wkT[:, kt, :],
                                     supT_hi[:, wi, kt, :], start=st, stop=sp)
                    nc.tensor.matmul(ps_lo[wi], wkT[:, kt, :],
                                     supT_lo[:, wi, kt, :], start=st, stop=sp)
            k_part = kevac.tile([128, 576], BF16, tag="kpart")
            for wi in range(3):
                nc.scalar.copy(k_part[:, wi * FEAT:wi * FEAT + 128], ps_hi[wi])
                nc.scalar.copy(k_part[:, wi * FEAT + 128:(wi + 1) * FEAT],
                               ps_lo[wi])
            nc.scalar.dma_start(
                _ap(kag_in.ap(), 0, [[576, 128], [1, 576]]), k_part)

        if not skip_cc:
            nc.gpsimd.collective_compute(
                "AllGather", mybir.AluOpType.bypass,
                replica_groups=[list(range(NCORES))],
                ins=[kag_in.ap()], outs=[kag_out.ap()])

        # ============ WqT (scaled 1/TEMP, bf16, XBAR) + q projection =====
        with ExitStack() as ctxq:
            wqp = ctxq.enter_context(tc.tile_pool(name=f"wqp{R}", bufs=2))
            wqt = ctxq.enter_context(tc.tile_pool(name=f"wqt{R}", bufs=1))
            tps = ctxq.enter_context(tc.tile_pool(name=f"tps{R}", bufs=3, space="PSUM"))

            wqT = wqt.tile([128, 4, NKT, 128], BF16, tag="wqT")
            for pt in range(4):
                t = wqp.tile([128, PTS], F32, tag=f"wqn{pt % 2}", name=f"wqn{R}{pt}")
                nc.scalar.dma_start(t, Wq[128 * pt:128 * (pt + 1), :])
                t_b = wqp.tile([128, PTS], BF16, tag=f"wqb{pt % 2}",
                               name=f"wqb{R}{pt}")
                nc.scalar.mul(t_b, t, 1.0 / TEMP)
                _guard(nc.scalar, t_b[0:1, 0:4])
                nc.scalar.dma_start_transpose(wqT[:, pt, :, :], t_b)

            qn = ctxq.enter_context(tc.tile_pool(name=f"qn{R}", bufs=6))
            qtp = ctxq.enter_context(tc.tile_pool(name=f"qtp{R}", bufs=4))
            qps = ctxq.enter_context(tc.tile_pool(name=f"qps{R}", bufs=1, space="PSUM"))

            for bp in range(BL // 2):
                b0 = 2 * bp
                nat = []
                for b in (b0, b0 + 1):
                    hi = qn.tile([128, PTS], F32, tag="qhi", name=f"qhi{R}{b}")
                    lo = qn.tile([64, PTS], F32, tag="qlo", name=f"qlo{R}{b}")
                    nc.sync.dma_start(hi, query[b, 0:128, :])
                    nc.sync.dma_start(lo, query[b, 128:192, :])
                    nat.append((hi, lo))
                ps_q = [qps.tile([128, 2 * FEAT], F32, tag=f"q{pt}",
                                 name=f"psq{R}{bp}{pt}") for pt in range(4)]
                for kt in range(NKT):
                    ksl = slice(128 * kt, 128 * (kt + 1))
                    # pack 4 transposes (2 batches x hi/lo) into one bank;
                    # fp32 in, bf16 on evacuation
                    ps_t = tps.tile([128, 2 * FEAT], F32, tag="tp")
                    for bi in range(2):
                        hi, lo = nat[bi]
                        nc.tensor.transpose(
                            ps_t[:, bi * FEAT:bi * FEAT + 128], hi[:, ksl], ident)
                        nc.tensor.transpose(
                            ps_t[:, bi * FEAT + 128:bi * FEAT + 192],
                            lo[:, ksl], ident[0:64, 0:64])
                    qt2 = qtp.tile([128, 2 * FEAT], BF16, tag="qt2")
                    if kt % 2 == 0:
                        nc.vector.tensor_copy(qt2, ps_t)
                    else:
                        nc.scalar.copy(qt2, ps_t)
                    for pt in range(4):
                        nc.tensor.matmul(
                            ps_q[pt], wqT[:, pt, kt, :], qt2,
                            start=(kt == 0), stop=(kt == NKT - 1))
                # evac + stage into a2a_in
                # chunk ch layout [b_src, j, bl, d]; this store covers
                # partitions p=128pt+64h+8ch+bl -> (h, ch, bl), free (b-pair, d)
                for pt in range(4):
                    q_sb = qtp.tile([128, 2 * FEAT], BF16, tag="qsb")
                    if pt % 2 == 0:
                        nc.vector.tensor_copy(q_sb, ps_q[pt])
                    else:
                        nc.scalar.copy(q_sb, ps_q[pt])
                    for h in range(2):
                        j = 2 * pt + h
                        for bi in range(2):
                            eng = nc.gpsimd if (pt % 2) else nc.scalar
                            eng.dma_start(
                                _ap(a2a_in.ap(),
                                    (b0 + bi) * (64 * FEAT) + j * (8 * FEAT),
                                    [[QCH, NCORES], [FEAT, 8], [1, FEAT]]),
                                q_sb[64 * h:64 * h + 64,
                                     bi * FEAT:(bi + 1) * FEAT])

        # ====== kflat reassembly from AllGathered pieces (bf16) ======
        with ExitStack() as ctka:
            kap = ctka.enter_context(tc.tile_pool(name=f"kap{R}", bufs=2))
            for pt in range(4):
                a0 = kap.tile([128, 576], BF16, tag="a0")
                a1 = kap.tile([128, 384], BF16, tag="a1")
                nc.sync.dma_start(
                    a0, _ap(kag_out.ap(), pt * (128 * 576),
                            [[576, 128], [1, 576]]))
                nc.sync.dma_start(
                    a1, _ap(kag_out.ap(), (pt + 4) * (128 * 576),
                            [[576, 128], [1, 384]]))
                k_sb = kap.tile([128, WAY * FEAT], BF16, tag="ksb")
                nc.vector.tensor_copy(k_sb[:, FEAT:4 * FEAT], a0)
                nc.vector.tensor_copy(k_sb[:, 4 * FEAT:WAY * FEAT], a1)
                # way-0 mean in fp32 for precision
                k0a = kap.tile([128, FEAT], F32, tag="k0a")
                k0b = kap.tile([128, FEAT], F32, tag="k0b")
                nc.vector.tensor_add(k0a, a0[:, 0:FEAT], a0[:, FEAT:2 * FEAT])
                nc.vector.tensor_add(k0b, a0[:, 2 * FEAT:3 * FEAT],
                                     a1[:, 0:FEAT])
                nc.vector.tensor_add(k0a, k0a, k0b)
                nc.vector.tensor_add(k0a, k0a, a1[:, FEAT:2 * FEAT])
                nc.vector.tensor_scalar_mul(k_sb[:, 0:FEAT], k0a, 1.0 / NSUP)
                nc.gpsimd.dma_start(
                    _ap(kflat.ap(), (128 * pt) * FEAT,
                        [[FEAT, 128], [PROJ * FEAT, WAY], [1, FEAT]]),
                    k_sb)

        # ============ AllToAll q exchange ============
        if not skip_cc:
            nc.gpsimd.collective_compute(
                "AllToAll", mybir.AluOpType.bypass,
                replica_groups=[list(range(NCORES))],
                ins=[a2a_in.ap()], outs=[a2a_out.ap()])

        # ============ attention + softmax + PV ============
        with ExitStack() as ctxa:
            kfp = ctxa.enter_context(tc.tile_pool(name=f"kfp{R}", bufs=1))
            lhp = ctxa.enter_context(tc.tile_pool(name=f"lhp{R}", bufs=1))
            aps = ctxa.enter_context(tc.tile_pool(name=f"aps{R}", bufs=2, space="PSUM"))
            ep = ctxa.enter_context(tc.tile_pool(name=f"ep{R}", bufs=4))
            sp = ctxa.enter_context(tc.tile_pool(name=f"sp{R}", bufs=6))
            scp = ctxa.enter_context(tc.tile_pool(name=f"scp{R}", bufs=6))

            kf = []
            for kt4 in range(4):
                t_r = kfp.tile([128, NCOL], BF16, tag=f"kf{kt4}", name=f"kfr{R}{kt4}")
                nc.sync.dma_start(
                    t_r, _ap(kflat.ap(), kt4 * 128 * NCOL,
                             [[NCOL, 128], [1, NCOL]]))
                kf.append(t_r)
            # whole-block lhsT loads: [128 r, 1536 m] per kt4 (contiguous rows)
            lhs_r = []
            for kt4 in range(4):
                l_r = lhp.tile([128, BL * FEAT], BF16, tag=f"lhr{kt4}",
                               name=f"lhr{R}{kt4}")
                nc.scalar.dma_start(
                    l_r, _ap(a2a_out.ap(), kt4 * 16 * (64 * FEAT),
                             [[64 * FEAT, 16], [BL * FEAT, 8], [1, BL * FEAT]]))
                lhs_r.append(l_r)

            for mt in range(MT):
                ps_at = [aps.tile([128, 2 * FEAT], F32, tag=f"at{nch}",
                                  name=f"psat{R}{mt}{nch}") for nch in range(3)]
                msl = slice(128 * mt, 128 * (mt + 1))
                for kt4 in range(4):
                    for nch in range(3):
                        nc.tensor.matmul(
                            ps_at[nch], lhs_r[kt4][:, msl],
                            kf[kt4][:, 384 * nch:384 * (nch + 1)],
                            start=(kt4 == 0), stop=(kt4 == 3))

                # exp without max-subtraction: logits are O(7) here, exp is
                # safe in fp32 and the softmax quotient is unchanged
                e6 = ep.tile([128, NCOL], F32, tag="e6")
                sums = sp.tile([128, WAY], F32, tag="sums")
                for nch in range(3):
                    for s2 in range(2):
                        s = 2 * nch + s2
                        seg = ps_at[nch][:, s2 * FEAT:(s2 + 1) * FEAT]
                        nc.scalar.activation(
                            e6[:, s * FEAT:(s + 1) * FEAT], seg, ExpF,
                            accum_out=sums[:, s:s + 1])

                vb = sp.tile([128, FEAT], F32, tag="vb")
                nc.scalar.dma_start(
                    vb, _ap(vflat.ap(), mt * 4 * FEAT,
                            [[FEAT, 4], [0, 32], [1, FEAT]]))
                opre = sp.tile([128, WAY], F32, tag="opre")
                for s in range(WAY):
                    scr = scp.tile([128, FEAT], F32, tag="scr")
                    nc.vector.scalar_tensor_tensor(
                        out=scr, in0=e6[:, s * FEAT:(s + 1) * FEAT],
                        scalar=1.0, in1=vb, op0=MULT, op1=MULT,
                        accum_out=opre[:, s:s + 1])
                rec = sp.tile([128, WAY], F32, tag="rec")
                nc.vector.reciprocal(rec, sums)
                nc.vector.tensor_mul(opre, opre, rec)
                nc.scalar.dma_start(
                    _ap(opre_d.ap(), mt * 128 * WAY, [[WAY, 128], [1, WAY]]),
                    opre)

        # ============ output head: fc + residual + layernorm ============
        with ExitStack() as ctxo:
            fp = ctxo.enter_context(tc.tile_pool(name=f"fp{R}", bufs=1))
            fps = ctxo.enter_context(tc.tile_pool(name=f"fps{R}", bufs=2, space="PSUM"))

            wf_hi = fp.tile([128, FEAT], F32, tag="wfn")
            wf_lo = fp.tile([64, FEAT], F32, tag="wfn2")
            nc.sync.dma_start(wf_hi, Wf[0:128, :])
            nc.sync.dma_start(wf_lo, Wf[128:192, :])
            wfT_hi = fp.tile([128, FEAT], F32R, tag="wft")
            wfT_lo = fp.tile([64, FEAT], F32R, tag="wft2")
            op_sb = fp.tile([GL, FEAT], F32, tag="opsb")
            nc.sync.dma_start(op_sb, _ap(opre_d.ap(), 0, [[FEAT, GL], [1, FEAT]]))
            opT_hi = fp.tile([128, GL], F32R, tag="opt")
            opT_lo = fp.tile([64, GL], F32R, tag="opt2")
            for (dst, dcol, src) in (
                (wfT_hi, slice(0, 128), wf_hi[:, 0:128]),
                (wfT_lo, slice(0, 128), wf_hi[:, 128:192]),
                (wfT_hi, slice(128, 192), wf_lo[:, 0:128]),
                (wfT_lo, slice(128, 192), wf_lo[:, 128:192]),
                (opT_hi, slice(0, GL), op_sb[:, 0:128]),
                (opT_lo, slice(0, GL), op_sb[:, 128:192]),
            ):
                p_in, f_in = src.shape
                ps_t = fps.tile([128, 128], F32, tag="tp")
                nc.tensor.transpose(ps_t[:f_in, :p_in], src, ident[:p_in, :p_in])
                nc.vector.tensor_copy(dst[:f_in, dcol], ps_t[:f_in, :p_in])

            ps_o = fps.tile([GL, FEAT], F32, tag="o2")
            nc.tensor.matmul(ps_o, opT_hi, wfT_hi, start=True, stop=False)
            nc.tensor.matmul(ps_o, opT_lo, wfT_lo, start=False, stop=True)

            x_sb = fp.tile([GL, FEAT], F32, tag="x")
            nc.vector.tensor_add(x_sb, ps_o, proto_sb)
            st = fp.tile([GL, 6], F32, tag="st")
            nc.vector.bn_stats(st, x_sb)
            mv = fp.tile([GL, 2], F32, tag="mv")
            nc.vector.bn_aggr(mv, st)
            eps_t = fp.tile([GL, 1], F32, tag="eps")
            nc.vector.memset(eps_t, LN_EPS)
            std = fp.tile([GL, 1], F32, tag="std")
            nc.scalar.activation(std, mv[:, 1:2], SqrtF, bias=eps_t, scale=1.0)
            rstd = fp.tile([GL, 1], F32, tag="rstd")
            nc.vector.reciprocal(rstd, std)
            negmean = fp.tile([GL, 1], F32, tag="nm")
            nc.scalar.mul(negmean, mv[:, 0:1], -1.0)
            y = fp.tile([GL, FEAT], F32, tag="y")
            nc.vector.tensor_scalar(
                out=y, in0=x_sb, scalar1=negmean, scalar2=rstd,
                op0=ADD, op1=MULT)
            lng_bc = fp.tile([GL, FEAT], F32, tag="lg")
            lnb_bc = fp.tile([GL, FEAT], F32, tag="lb")
            nc.scalar.dma_start(lng_bc, _ap(ln_g, 0, [[0, GL], [1, FEAT]]))
            nc.scalar.dma_start(lnb_bc, _ap(ln_b, 0, [[0, GL], [1, FEAT]]))
            nc.vector.tensor_mul(y, y, lng_bc)
            nc.vector.tensor_add(y, y, lnb_bc)
            nc.sync.dma_start(out_l, y)


_NC = None


def kernel(query, support, prototype, Wq, Wk, Wv, bv, Wf, ln_g, ln_b,
           _trace=False):
    global _NC
    if _NC is None:
        _NC = build()
    query = np.ascontiguousarray(np.asarray(query, np.float32))
    prototype = np.ascontiguousarray(np.asarray(prototype, np.float32))
    support = np.asarray(support, np.float32)
    Wk = np.asarray(Wk, np.float32)
    shared = {
        "Wq": np.ascontiguousarray(np.asarray(Wq, np.float32)),
        "Wv": np.ascontiguousarray(np.asarray(Wv, np.float32)),
        "bv": np.ascontiguousarray(np.asarray(bv, np.float32)),
        "Wf": np.ascontiguousarray(np.asarray(Wf, np.float32)),
        "ln_g": np.ascontiguousarray(np.asarray(ln_g, np.float32)),
        "ln_b": np.ascontiguousarray(np.asarray(ln_b, np.float32)),
    }
    zway = np.zeros((1, FEAT, PTS), np.float32)
    sup_h0 = np.ascontiguousarray(support[0:3])
    sup_h1 = np.ascontiguousarray(np.concatenate([support[3:5], zway], 0))
    pf = prototype.reshape(BATCH * WAY, FEAT)
    in_maps = []
    for c in range(NCORES):
        pt, half = c % 4, c // 4
        in_maps.append({
            "query": np.ascontiguousarray(query[c * BL:(c + 1) * BL]),
            "proto": np.ascontiguousarray(pf[c * GL:(c + 1) * GL]),
            "sup_sh": sup_h0 if half == 0 else sup_h1,
            "Wk_sh": np.ascontiguousarray(Wk[128 * pt:128 * (pt + 1)]),
            **shared,
        })
    res = run_bass_kernel_spmd(_NC, in_maps, list(range(NCORES)),
                               trace=_trace)
    out = np.concatenate([res.results[c]["out_l"] for c in range(NCORES)], 0)
    out = out.reshape(BATCH, WAY, FEAT)
    if _trace:
        return out, res
    return out
